# revision 1
# baseline (speedup 1.0000x reference)
"""Sparse multi-head attention (nn_MultiHeadAttention_44332652429419) on 8 trn2 cores.

Strategy (tensor-parallel over H=16 heads, 2 heads per core):
  Host: compose the two stacked linear layers (q/k/v_proj followed by
        MultiheadAttention in_proj) into one weight per tensor; build the
        dense multiplicative mask exp(additive_mask) transposed; transpose x.
  Device (per core, SPMD with per-core weight slices):
    q2T/k2T/v2T = W_c @ x.T + b_c           [128, 3072] (2 heads x 64 dims)
    scoresT[mk,nq] = k2T_h.T-slice @ q2T_h  (K=64, two heads row-packed in PE)
    P = exp(scoresT * 1/8) * maskT          (ACT exp from PSUM, DVE multiply)
    outT_aug = [v_h | 1].T @ P              (rowsum via ones-augmented V)
    attnT = outT[:64] / outT[64]            (DVE recip + partition broadcast)
    ypart = attnT.T-slices @ woT_c          (partial out_proj, K=128)
  Host: y = sum_c ypart_c + bo
"""
import os
import sys

sys.path.insert(0, "/opt/trn_rl_repo")

import numpy as np
from contextlib import ExitStack

import concourse.bass as bass
import concourse.bacc as bacc
import concourse.mybir as mybir
import concourse.tile as tile
from concourse.bass_utils import run_bass_kernel_spmd
from concourse.masks import make_identity

F32 = mybir.dt.float32
F32R = mybir.dt.float32r
BF16 = mybir.dt.bfloat16
F16 = mybir.dt.float16
AF = mybir.ActivationFunctionType
ALU = mybir.AluOpType

N = 3072
IN_F = 1024
OUT_F = 1024
H = 16
D = 64
NCORES = 8
HPC = H // NCORES            # heads per core = 2
CW = HPC * D                 # per-core width = 128
P = 128
NT = N // P                  # 24 node tiles
KT = IN_F // P               # 8 contraction tiles
SQ = 1024                    # query strip width (phase B)
NSQ = N // SQ                # 3 strips
SP = 512                     # proj strip width (phase A)
NSP = N // SP                # 6 strips
SCALE = 1.0 / 8.0            # 1/sqrt(D)

MASK_DT = F32R               # additive mask, pre-scaled by 1/SCALE
MASK_ALL_PE = False          # apply mask via PE identity-add for all tiles


def build_program():
    nc = bacc.Bacc()
    xT = nc.declare_dram_parameter("xT", [IN_F, N], F32R, isOutput=False)
    # additive mask (pre-scaled by 1/SCALE) for even key tiles, multiplicative
    # exp-mask for odd key tiles — hybrid PE/DVE mask application
    maskA = nc.declare_dram_parameter("maskA", [N // 3, N], F16, isOutput=False)
    maskM = nc.declare_dram_parameter("maskM", [2 * N // 3, N], F16, isOutput=False)
    wqT = nc.declare_dram_parameter("wqT", [IN_F, CW], F32R, isOutput=False)
    wkT = nc.declare_dram_parameter("wkT", [IN_F, CW], F32R, isOutput=False)
    wvT = nc.declare_dram_parameter("wvT", [IN_F, CW], F32R, isOutput=False)
    bq = nc.declare_dram_parameter("bq", [CW], F32, isOutput=False)
    bk = nc.declare_dram_parameter("bk", [CW], F32, isOutput=False)
    bv = nc.declare_dram_parameter("bv", [CW], F32, isOutput=False)
    woT = nc.declare_dram_parameter("woT", [CW, OUT_F], F32R, isOutput=False)
    ypart = nc.declare_dram_parameter("ypart", [N, OUT_F], F32, isOutput=True)

    with tile.TileContext(nc) as tc, ExitStack() as ctx:
        cst = ctx.enter_context(tc.tile_pool(name="cst", bufs=1))
        lp = ctx.enter_context(tc.tile_pool(name="lp", bufs=2))       # xs/v2Ts
        wp = ctx.enter_context(tc.tile_pool(name="wp", bufs=3))       # loop tiles
        pq = ctx.enter_context(tc.tile_pool(name="pq", bufs=5))       # p/pm queue
        ep = ctx.enter_context(tc.tile_pool(name="ep", bufs=1))       # epilogue
        pp = ctx.enter_context(tc.tile_pool(name="pp", bufs=2, space="PSUM"))
        pso = ctx.enter_context(tc.tile_pool(name="pso", bufs=1, space="PSUM"))

        ident = cst.tile([P, P], F32)
        make_identity(nc, ident)
        identR = cst.tile([P, P], F32R)
        nc.vector.tensor_copy(identR[:], ident[:])
        identH = cst.tile([P, P], F16)
        nc.vector.tensor_copy(identH[:], ident[:])

        # per-512-strip persistent tensors (fine-grained deps let phase B
        # start while projections still run)
        q2s = [cst.tile([P, SP], F32R, tag=f"q2s{s}", name=f"q2s{s}")
               for s in range(NSP)]
        # k2z[h][s]: only rows h*D..h*D+63 live, rest zero — score matmuls
        # contract over full K=128 (keeps the PE HAM activity monitor warm)
        k2zs = [[cst.tile([P, SP], F32R, tag=f"k2z{h}_{s}", name=f"k2z{h}_{s}")
                 for s in range(NSP)] for h in range(HPC)]
        attn_t = [cst.tile([P, P], F32R, tag=f"attn{t}", name=f"attn{t}")
                  for t in range(NT)]
        vaug = [cst.tile([P, NT, D + 1], F32R, tag=f"vaug{h}", name=f"vaug{h}")
                for h in range(HPC)]
        ones_col = cst.tile([P, 1], F32)
        nc.vector.memset(ones_col[:], 1.0)
        zero_col = cst.tile([P, 1], F32)
        nc.vector.memset(zero_col[:], 0.0)
        for h in range(HPC):
            nc.vector.tensor_copy(vaug[h][:, :, D:D + 1],
                                  ones_col[:, 0:1, None].to_broadcast([P, NT, 1]))
            osl = slice((1 - h) * D, (2 - h) * D)   # the dead half of k2z[h]
            for s in range(NSP):
                nc.vector.tensor_copy(k2zs[h][s][osl, :],
                                      zero_col[osl, 0:1].to_broadcast([D, SP]))

        # weights
        wq_sb = cst.tile([P, KT, CW], F32R)
        nc.sync.dma_start(wq_sb[:], wqT.rearrange("(k p) m -> p k m", p=P))
        wk_sb = cst.tile([P, KT, CW], F32R)
        nc.sync.dma_start(wk_sb[:], wkT.rearrange("(k p) m -> p k m", p=P))
        wv_sb = cst.tile([P, KT, CW], F32R)
        nc.sync.dma_start(wv_sb[:], wvT.rearrange("(k p) m -> p k m", p=P))
        wo_sb = cst.tile([P, OUT_F], F32R)
        nc.sync.dma_start(wo_sb[:], woT[:])
        bq_sb = cst.tile([P, 1], F32)
        nc.sync.dma_start(bq_sb[:], bq[:, None])
        bk_sb = cst.tile([P, 1], F32)
        nc.sync.dma_start(bk_sb[:], bk[:, None])
        bv_sb = cst.tile([P, 1], F32)
        nc.sync.dma_start(bv_sb[:], bv[:, None])

        # ---- emission helpers ----
        def emit_proj_strip(s):
            xs_a = lp.tile([P, KT // 2, SP], F32R, tag="xs_a", name="xs_a")
            nc.sync.dma_start(
                xs_a[:], xT.rearrange("(k p) n -> p k n", p=P)[:, 0:KT // 2,
                                                              s * SP:(s + 1) * SP])
            xs_b = lp.tile([P, KT // 2, SP], F32R, tag="xs_b", name="xs_b")
            nc.sync.dma_start(
                xs_b[:], xT.rearrange("(k p) n -> p k n", p=P)[:, KT // 2:KT,
                                                               s * SP:(s + 1) * SP])
            def xsk(k):
                return xs_a[:, k, :] if k < KT // 2 else xs_b[:, k - KT // 2, :]
            ps = pp.tile([P, SQ], F32, tag="ps_s", name="ps_q")
            for k in range(KT):
                nc.tensor.matmul(ps[:, 0:SP], wq_sb[:, k, :], xsk(k),
                                 start=(k == 0), stop=(k == KT - 1))
            nc.vector.tensor_scalar_add(q2s[s][:], ps[:, 0:SP], bq_sb[:, 0:1])
            ps = pp.tile([P, SQ], F32, tag="ps_s", name="ps_k")
            for k in range(KT):
                nc.tensor.matmul(ps[:, 0:SP], wk_sb[:, k, :], xsk(k),
                                 start=(k == 0), stop=(k == KT - 1))
            for h in range(HPC):
                hsl = slice(h * D, (h + 1) * D)
                nc.vector.tensor_scalar_add(k2zs[h][s][hsl, :], ps[hsl, 0:SP],
                                            bk_sb[hsl, 0:1])
            # v: project then transpose into vaug
            ps = pp.tile([P, SQ], F32, tag="ps_s", name="ps_v")
            for k in range(KT):
                nc.tensor.matmul(ps[:, 0:SP], wv_sb[:, k, :], xsk(k),
                                 start=(k == 0), stop=(k == KT - 1))
            v2Ts = lp.tile([P, SP], F32, tag="v2Ts", name="v2Ts")
            nc.vector.tensor_scalar_add(v2Ts[:], ps[:, 0:SP], bv_sb[:, 0:1])
            for b in range(SP // P):
                t = s * (SP // P) + b
                ps_t = pp.tile([P, SQ], F32, tag="ps_s", name="ps_t")
                nc.tensor.transpose(ps_t[:, 0:P], v2Ts[:, b * P:(b + 1) * P],
                                    ident[:])
                for h in range(HPC):
                    nc.vector.tensor_copy(vaug[h][:, t, 0:D],
                                          ps_t[:, h * D:h * D + D])

        def emit_pv(ps_o, h, mk, p):
            for half in range(SQ // SP):
                fsl = slice(half * SP, (half + 1) * SP)
                nc.tensor.matmul(
                    ps_o[h][:, fsl],
                    vaug[h][:, mk, :],
                    p[:, fsl],
                    start=(mk == 0), stop=(mk == NT - 1),
                )

        def emit_attn_tiles(sq, ps_o, mks, pend):
            for mk in mks:
                use_pe = (mk % 3 == 0) or MASK_ALL_PE
                if use_pe:
                    blk = mk // 3
                    mt = wp.tile([P, SQ], F16, tag="mta", name="mta")
                    nc.sync.dma_start(
                        mt[:], maskA[blk * P:(blk + 1) * P,
                                     sq * SQ:(sq + 1) * SQ])
                else:
                    blk = mk - mk // 3 - 1
                    mt = wp.tile([P, SQ], F16, tag="mtm", name="mtm")
                    nc.sync.dma_start(
                        mt[:], maskM[blk * P:(blk + 1) * P,
                                     sq * SQ:(sq + 1) * SQ])
                for h in range(HPC):
                    ps_s = pp.tile([P, SQ], F32, tag="ps_s", name="ps_s")
                    for half in range(SQ // SP):
                        fsl = slice(half * SP, (half + 1) * SP)
                        nc.tensor.matmul(
                            ps_s[:, fsl],
                            k2zs[h][mk // 4][:, (mk % 4) * P:(mk % 4 + 1) * P],
                            q2s[sq * (SQ // SP) + half][:],
                            start=True, stop=not use_pe,
                        )
                        if use_pe:
                            nc.tensor.matmul(
                                ps_s[:, fsl], identH[:], mt[:, fsl],
                                start=False, stop=True,
                            )
                    p = pq.tile([P, SQ], F32R, tag="p", name="p")
                    nc.scalar.activation(p[:], ps_s[:], AF.Exp, scale=SCALE)
                    if not use_pe:
                        pm = pq.tile([P, SQ], F32R, tag="pm", name="pm")
                        nc.vector.tensor_tensor(pm[:], p[:], mt[:], ALU.mult)
                        p = pm
                    # software-pipeline: defer this tile's PV until after the
                    # next tile's scores so the PE stream never head-of-line
                    # blocks on the exp
                    pend.append((h, mk, p))
                    if len(pend) > 3:
                        emit_pv(ps_o, *pend.pop(0))

        def emit_epilogue(sq, ps_o):
            # stage PSUM accumulators to SBUF immediately so the next strip's
            # PV matmuls get the banks back as early as possible
            osb, bcs = [], []
            for h in range(HPC):
                ob = ep.tile([D + 1, SQ], F32, tag=f"osb{h}", name=f"osb{h}")
                nc.vector.tensor_copy(ob[:], ps_o[h][:])
                osb.append(ob)
            for h in range(HPC):
                zrow = ep.tile([1, SQ], F32, tag="zrow", name=f"zrow{h}")
                nc.vector.tensor_copy(zrow[:], osb[h][D:D + 1, :])
                recip = ep.tile([1, SQ], F32, tag="recip", name=f"recip{h}")
                nc.vector.reciprocal_approx_fast(recip[:], zrow[:])
                bc = ep.tile([D, SQ], F32, tag=f"bc{h}", name=f"bc{h}")
                nc.gpsimd.partition_broadcast(bc[:], recip[:])
                bcs.append(bc)
            # normalize per node tile so out_proj starts early
            for b in range(SQ // P):
                t = sq * (SQ // P) + b
                for h in range(HPC):
                    nc.vector.tensor_tensor(
                        attn_t[t][h * D:(h + 1) * D, :],
                        osb[h][0:D, b * P:(b + 1) * P],
                        bcs[h][:, b * P:(b + 1) * P], ALU.mult)
                ps_y = pp.tile([P, SQ], F32, tag="ps_s", name="ps_y")
                for f in range(OUT_F // 512):
                    nc.tensor.matmul(ps_y[:, f * 512:(f + 1) * 512],
                                     attn_t[t][:, :],
                                     wo_sb[:, f * 512:(f + 1) * 512],
                                     start=True, stop=True)
                ys = wp.tile([P, OUT_F], F32, tag="ys", name="ys")
                nc.vector.tensor_copy(ys[:], ps_y[:])
                nc.sync.dma_start(ypart[t * P:(t + 1) * P, :], ys[:])

        # ---- interleaved emission: A strips feed B(sq=0) chunks ----
        emit_proj_strip(0)
        emit_proj_strip(1)
        ps_o0 = [pso.tile([D + 1, SQ], F32, tag=f"ps_o{h}", name=f"ps_o{h}")
                 for h in range(HPC)]
        pend0 = []
        emit_attn_tiles(0, ps_o0, range(0, 8), pend0)
        emit_proj_strip(2)
        emit_attn_tiles(0, ps_o0, range(8, 12), pend0)
        emit_proj_strip(3)
        emit_attn_tiles(0, ps_o0, range(12, 16), pend0)
        emit_proj_strip(4)
        emit_attn_tiles(0, ps_o0, range(16, 20), pend0)
        emit_proj_strip(5)
        emit_attn_tiles(0, ps_o0, range(20, 24), pend0)
        ps_o_cur, pend_cur = ps_o0, pend0
        for sq in range(1, NSQ):
            ps_o_nxt = [pso.tile([D + 1, SQ], F32, tag=f"ps_o{h}",
                                 name=f"ps_o{h}_{sq}") for h in range(HPC)]
            pend_nxt = []
            emit_attn_tiles(sq, ps_o_nxt, range(0, 4), pend_nxt)
            for args in pend_cur:
                emit_pv(ps_o_cur, *args)
            emit_attn_tiles(sq, ps_o_nxt, range(4, 8), pend_nxt)
            emit_epilogue(sq - 1, ps_o_cur)
            emit_attn_tiles(sq, ps_o_nxt, range(8, NT), pend_nxt)
            ps_o_cur, pend_cur = ps_o_nxt, pend_nxt
        for args in pend_cur:
            emit_pv(ps_o_cur, *args)
        emit_epilogue(NSQ - 1, ps_o_cur)

    nc.compile()
    return nc


_PROGRAM = None
LAST_RESULTS = None


def _get_program():
    global _PROGRAM
    if _PROGRAM is None:
        _PROGRAM = build_program()
    return _PROGRAM


def _softplus(x):
    x = np.asarray(x, np.float32)
    return np.logaddexp(0.0, x).astype(np.float32)


def host_prep(inputs):
    x = np.asarray(inputs["x"], np.float32)
    edge_index = np.asarray(inputs["edge_index"])
    edge_type = np.asarray(inputs["edge_type"])
    etw = np.asarray(inputs["edge_type_weights"], np.float32)

    def f32(k):
        return np.asarray(inputs[k], np.float32)

    # compose the two linear layers: q2 = x @ (wiq@wq).T + (wiq@bq + biq)
    WQ = f32("wiq") @ f32("wq")
    bQ = f32("wiq") @ f32("bq") + f32("biq")
    WK = f32("wik") @ f32("wk")
    bK = f32("wik") @ f32("bk") + f32("bik")
    WV = f32("wiv") @ f32("wv")
    bV = f32("wiv") @ f32("bv") + f32("biv")
    wo = f32("wo")
    bo = f32("bo")

    # multiplicative mask, transposed: maskT[m, n] = exp(add_mask[n, m])
    w = _softplus(etw)
    NEG = np.float32(-60000.0)
    M = np.full((N, N), NEG, dtype=np.float32)
    src, dst = edge_index[0], edge_index[1]
    wv8 = (w * np.float32(1.0 / SCALE)).astype(np.float32)
    M[src, dst] = wv8[edge_type - 1]           # last write wins, like jax .at[].set
    diag = np.diagonal(M).copy()
    didx = np.arange(N)
    M[didx, didx] = np.where(diag == NEG, wv8[3], diag)
    MT = np.ascontiguousarray(M.T)             # [key m, query n], additive * 8
    # even key tiles use the additive form on the PE, odd tiles the
    # multiplicative exp-form on the DVE
    MT4 = MT.reshape(NT, P, N)
    pe_rows = MT4[0::3]
    dve_rows = np.concatenate([MT4[1::3], MT4[2::3]])
    # interleave dve blocks back in mk order: mk%3==1,2 -> positions
    order = [mk for mk in range(NT) if mk % 3 != 0]
    dve_sorted = np.empty((len(order), P, N), np.float32)
    pos = {mk: i for i, mk in enumerate(sorted(order))}
    srcs = [mk for mk in range(NT) if mk % 3 == 1] + \
           [mk for mk in range(NT) if mk % 3 == 2]
    for i, mk in enumerate(srcs):
        dve_sorted[pos[mk]] = dve_rows[i]
    maskA = pe_rows.reshape(N // 3, N).astype(np.float16)
    maskM = np.exp(dve_sorted.reshape(2 * N // 3, N).astype(np.float64)
                   * np.float64(SCALE)).astype(np.float16)

    xT = np.ascontiguousarray(x.T)

    in_maps = []
    for c in range(NCORES):
        rs = slice(c * CW, (c + 1) * CW)
        in_maps.append({
            "xT": xT,
            "maskA": maskA,
            "maskM": maskM,
            "wqT": np.ascontiguousarray(WQ[rs].T),
            "wkT": np.ascontiguousarray(WK[rs].T),
            "wvT": np.ascontiguousarray(WV[rs].T),
            "bq": np.ascontiguousarray(bQ[rs]),
            "bk": np.ascontiguousarray(bK[rs]),
            "bv": np.ascontiguousarray(bV[rs]),
            "woT": np.ascontiguousarray(wo[:, rs].T),
        })
    return in_maps, bo


def kernel(**inputs) -> np.ndarray:
    global LAST_RESULTS
    in_maps, bo = host_prep(inputs)
    nc = _get_program()
    trace = bool(os.environ.get("KERNEL_TRACE"))
    res = run_bass_kernel_spmd(nc, in_maps, list(range(NCORES)), trace=trace)
    LAST_RESULTS = res
    y = bo[None, :].astype(np.float32).repeat(N, axis=0)
    for c in range(NCORES):
        y += res.results[c]["ypart"]
    return y



# revision 23
# speedup vs baseline: 1.2314x; 1.2314x over previous
"""Sparse multi-head attention (nn_MultiHeadAttention_44332652429419) on 8 trn2 cores.

Strategy v2 (tensor-parallel over H=16 heads, 2 heads per core, all-bf16 PE):
  Host: compose the stacked linears into one weight per tensor (bf16);
        drop bk (cancels in softmax) and bv (folds into the output bias,
        since attention rows sum to 1); build the multiplicative mask
        exp(additive) transposed, f16; transpose x (bf16).
  Device (per core, SPMD with per-core weight slices):
    q2T/k2T/v2T = W_c @ x.T (+ bq via ones-row)   [128, 3072] bf16
    per (strip of 512 queries, key tile mk):
      scoresT[h] = k2z_h.T @ q2s  -> paired PSUM [128, 2, 512]
      p = exp(scores * 1/8)       one ACT op over both heads (1024 free)
      pm = p * maskT tile         one DVE op, 16-bit (mask broadcast over h)
      ps_o[h] += vaug_h @ pm_h    PV accumulation over mk (ones-col rowsum)
    epilogue per strip: normalize by rowsum, out_proj partial -> ypart bf16
  Host: y = sum_c ypart_c + (bv_eff @ wo.T + bo)
"""
import os
import sys

sys.path.insert(0, "/opt/trn_rl_repo")

import numpy as np
import ml_dtypes
from contextlib import ExitStack

import concourse.bass as bass
import concourse.bacc as bacc
import concourse.mybir as mybir
import concourse.tile as tile
from concourse.bass_utils import run_bass_kernel_spmd
from concourse.masks import make_identity

F32 = mybir.dt.float32
F32R = mybir.dt.float32r
BF16 = mybir.dt.bfloat16
F16 = mybir.dt.float16
AF = mybir.ActivationFunctionType
ALU = mybir.AluOpType

N = 3072
IN_F = 1024
OUT_F = 1024
H = 16
D = 64
NCORES = 8
HPC = H // NCORES            # heads per core = 2
CW = HPC * D                 # per-core width = 128
P = 128
NT = N // P                  # 24 key tiles
KT = IN_F // P               # 8 contraction tiles
SP = 512                     # strip width (queries/nodes)
NSP = N // SP                # 6 strips
SCALE = 1.0 / 8.0            # 1/sqrt(D)


def build_program():
    nc = bacc.Bacc()
    xT = nc.declare_dram_parameter("xT", [IN_F, N], BF16, isOutput=False)
    maskT = nc.declare_dram_parameter("maskT", [N, N], F16, isOutput=False)
    wqT = nc.declare_dram_parameter("wqT", [IN_F, CW], BF16, isOutput=False)
    wkT = nc.declare_dram_parameter("wkT", [IN_F, CW], BF16, isOutput=False)
    wvT = nc.declare_dram_parameter("wvT", [IN_F, CW], BF16, isOutput=False)
    bq = nc.declare_dram_parameter("bq", [1, CW], BF16, isOutput=False)
    woT = nc.declare_dram_parameter("woT", [CW, OUT_F], F32R, isOutput=False)
    ypart = nc.declare_dram_parameter("ypart", [N, OUT_F], BF16, isOutput=True)

    with tile.TileContext(nc) as tc, ExitStack() as ctx:
        cst = ctx.enter_context(tc.tile_pool(name="cst", bufs=1))
        lp = ctx.enter_context(tc.tile_pool(name="lp", bufs=3))       # xs strips
        mtp = ctx.enter_context(tc.tile_pool(name="mtp", bufs=8))     # mask tiles
        ppq = ctx.enter_context(tc.tile_pool(name="ppq", bufs=4))     # p/pm pairs
        ep = ctx.enter_context(tc.tile_pool(name="ep", bufs=2))       # epilogue
        # PSUM: spair 2x2 banks + pso 2 banks + pwork 2x1 banks = 8
        spair = ctx.enter_context(tc.tile_pool(name="spair", bufs=2, space="PSUM"))
        pso = ctx.enter_context(tc.tile_pool(name="pso", bufs=1, space="PSUM"))
        pwork = ctx.enter_context(tc.tile_pool(name="pwork", bufs=2, space="PSUM"))

        ident = cst.tile([P, P], F32)
        make_identity(nc, ident)

        ones_row = cst.tile([1, SP], BF16)
        nc.vector.memset(ones_row[:], 1.0)
        zero_col = cst.tile([P, 1], BF16)
        nc.vector.memset(zero_col[:], 0.0)
        one_col = cst.tile([P, 1], BF16)
        nc.vector.memset(one_col[:], 1.0)

        # persistent activations (bf16)
        q2s = [cst.tile([P, SP], BF16, tag=f"q2s{s}", name=f"q2s{s}")
               for s in range(NSP)]
        k2zs = [[cst.tile([P, SP], BF16, tag=f"k2z{h}_{s}", name=f"k2z{h}_{s}")
                 for s in range(NSP)] for h in range(HPC)]
        vaug = [cst.tile([P, NT, D + 1], BF16, tag=f"vaug{h}", name=f"vaug{h}")
                for h in range(HPC)]
        attn_s = [cst.tile([P, SP], F32R, tag=f"attn{s}", name=f"attn{s}")
                  for s in range(NSP)]
        for h in range(HPC):
            nc.vector.tensor_copy(vaug[h][:, :, D:D + 1],
                                  one_col[:, 0:1, None].to_broadcast([P, NT, 1]))
            osl = slice((1 - h) * D, (2 - h) * D)   # dead half of k2z[h]
            for s in range(NSP):
                nc.vector.tensor_copy(k2zs[h][s][osl, :],
                                      zero_col[osl, 0:1].to_broadcast([D, SP]))

        # weights
        wq_sb = cst.tile([P, KT, CW], BF16)
        for _h2 in range(2):
            nc.sync.dma_start(
                wq_sb[:, 4 * _h2:4 * _h2 + 4, :],
                wqT.rearrange("(k p) m -> p k m", p=P)[:, 4 * _h2:4 * _h2 + 4, :])
        wk_sb = cst.tile([P, KT, CW], BF16)
        for _h2 in range(2):
            nc.sync.dma_start(
                wk_sb[:, 4 * _h2:4 * _h2 + 4, :],
                wkT.rearrange("(k p) m -> p k m", p=P)[:, 4 * _h2:4 * _h2 + 4, :])
        wv_sb = cst.tile([P, KT, CW], BF16)
        for _h2 in range(2):
            nc.sync.dma_start(
                wv_sb[:, 4 * _h2:4 * _h2 + 4, :],
                wvT.rearrange("(k p) m -> p k m", p=P)[:, 4 * _h2:4 * _h2 + 4, :])
        wo_sb = cst.tile([P, OUT_F], F32R)
        bq_sb = cst.tile([1, CW], BF16)
        nc.sync.dma_start(bq_sb[:], bq[:])

        # ---- phase A: k/v projections (needed progressively by all strips);
        #      q-proj is per-strip and deferrable to just before B(s) ----
        def emit_q_strip(s, xs_reuse=None):
            if xs_reuse is None:
                xq_c = [lp.tile([P, 2, SP], BF16, tag=f"xq_c{c}", name=f"xq_c{c}")
                        for c in range(KT // 2)]
                for c in range(KT // 2):
                    nc.sync.dma_start(
                        xq_c[c][:],
                        xT.rearrange("(k p) n -> p k n", p=P)[:, 2 * c:2 * c + 2,
                                                              s * SP:(s + 1) * SP])
            else:
                xq_c = xs_reuse
            ps = pwork.tile([P, SP], F32, tag="pw", name="ps_q")
            for k in range(KT):
                nc.tensor.matmul(ps[:], wq_sb[:, k, :], xq_c[k // 2][:, k % 2, :],
                                 start=(k == 0), stop=False)
            nc.tensor.matmul(ps[:], bq_sb[:], ones_row[:],
                             start=False, stop=True)
            nc.vector.tensor_copy(q2s[s][:], ps[:])

        def emit_kv_strip(s):
            xs_c = [lp.tile([P, 2, SP], BF16, tag=f"xs_c{c}", name=f"xs_c{c}")
                    for c in range(KT // 2)]
            for c in range(KT // 2):
                if s == 0:
                    for j in range(2):
                        nc.sync.dma_start(
                            xs_c[c][:, j, :],
                            xT.rearrange("(k p) n -> p k n", p=P)[:, 2 * c + j,
                                                                  0:SP])
                else:
                    nc.sync.dma_start(
                        xs_c[c][:],
                        xT.rearrange("(k p) n -> p k n", p=P)[:, 2 * c:2 * c + 2,
                                                              s * SP:(s + 1) * SP])

            def xsk(k):
                return xs_c[k // 2][:, k % 2, :]

            # k-proj (no bias)
            ps = pwork.tile([P, SP], F32, tag="pw", name="ps_k")
            for k in range(KT):
                nc.tensor.matmul(ps[:], wk_sb[:, k, :], xsk(k),
                                 start=(k == 0), stop=(k == KT - 1))
            for h in range(HPC):
                hsl = slice(h * D, (h + 1) * D)
                nc.vector.tensor_copy(k2zs[h][s][hsl, :], ps[hsl, :])
            # v-proj (no bias), then transpose into vaug
            ps = pwork.tile([P, SP], F32, tag="pw", name="ps_v")
            for k in range(KT):
                nc.tensor.matmul(ps[:], wv_sb[:, k, :], xsk(k),
                                 start=(k == 0), stop=(k == KT - 1))
            v2Ts = lp.tile([P, SP], F32, tag="v2Ts", name="v2Ts")
            nc.vector.tensor_copy(v2Ts[:], ps[:])
            for b in range(SP // P):
                t = s * (SP // P) + b
                ps_t = pwork.tile([P, SP], F32, tag="pw", name="ps_t")
                nc.tensor.transpose(ps_t[:, 0:P], v2Ts[:, b * P:(b + 1) * P],
                                    ident[:])
                for h in range(HPC):
                    nc.vector.tensor_copy(vaug[h][:, t, 0:D],
                                          ps_t[:, h * D:h * D + D])
            return xs_c

        # ---- phase B: one (strip, key-tile) step, PV deferred via pend ----
        def emit_pv(ps_o_t, mk, pm):
            for h in range(HPC):
                nc.tensor.matmul(ps_o_t[h][:], vaug[h][:, mk, :], pm[:, h, :],
                                 start=(mk == 0), stop=(mk == NT - 1))

        def emit_B(s, mk, pend):
            mt = mtp.tile([P, SP], F16, tag="mt", name="mt")
            nc.sync.dma_start(mt[:], maskT[mk * P:(mk + 1) * P,
                                           s * SP:(s + 1) * SP])
            sp_ = spair.tile([P, HPC, SP], F32, tag="sp", name="sp")
            for h in range(HPC):
                nc.tensor.matmul(sp_[:, h, :],
                                 k2zs[h][mk // 4][:, (mk % 4) * P:(mk % 4 + 1) * P],
                                 q2s[s][:], start=True, stop=True)
            p_ = ppq.tile([P, HPC, SP], BF16, tag="p", name="p")
            nc.scalar.activation(p_[:], sp_[:], AF.Exp, scale=SCALE)
            pm = ppq.tile([P, HPC, SP], BF16, tag="pm", name="pm")
            nc.vector.tensor_tensor(pm[:], p_[:],
                                    mt[:, None, :].to_broadcast([P, HPC, SP]),
                                    ALU.mult)
            pend.append((ps_o, mk, pm))
            if len(pend) > 2:
                emit_pv(*pend.pop(0))

        # ---- epilogue, split: head frees PSUM early; tail does out_proj ----
        def emit_ep_head(s, ps_o_s):
            osbs = []
            for h in range(HPC):
                osb = ep.tile([D + 1, SP], F32, tag=f"osb{h}", name=f"osb{h}")
                nc.vector.tensor_copy(osb[:], ps_o_s[h][:])
                osbs.append(osb)
            parts = []
            for h in range(HPC):
                osb = osbs[h]
                zrow = ep.tile([1, SP], F32, tag="zrow", name=f"zrow{h}")
                nc.vector.tensor_copy(zrow[:], osb[D:D + 1, :])
                recip = ep.tile([1, SP], F32, tag="recip", name=f"recip{h}")
                nc.vector.reciprocal_approx_fast(recip[:], zrow[:])
                bc = ep.tile([D, SP], F32, tag=f"bc{h}", name=f"bc{h}")
                nc.gpsimd.partition_broadcast(bc[:], recip[:])
                parts.append((osb, bc))
            return parts

        def emit_ep_tail(s, parts, last=False):
            for h in range(HPC):
                osb, bc = parts[h]
                nc.vector.tensor_tensor(attn_s[s][h * D:(h + 1) * D, :],
                                        osb[0:D, :], bc[:], ALU.mult)
            for b in range(SP // P):
                t = s * (SP // P) + b
                ys = ep.tile([P, OUT_F], BF16, tag="ys", name="ys")
                for f in range(OUT_F // SP):
                    ps_y = pwork.tile([P, SP], F32, tag="pw", name="ps_y")
                    nc.tensor.matmul(ps_y[:],
                                     attn_s[s][:, b * P:(b + 1) * P],
                                     wo_sb[:, f * SP:(f + 1) * SP],
                                     start=True, stop=True)
                    if (b + f) % 2 == 0:
                        nc.scalar.activation(ys[:, f * SP:(f + 1) * SP],
                                             ps_y[:], AF.Copy)
                    else:
                        nc.vector.tensor_copy(ys[:, f * SP:(f + 1) * SP],
                                              ps_y[:])
                    if last:
                        nc.sync.dma_start(
                            ypart[t * P:(t + 1) * P, f * SP:(f + 1) * SP],
                            ys[:, f * SP:(f + 1) * SP])
                if not last:
                    nc.sync.dma_start(ypart[t * P:(t + 1) * P, :], ys[:])

        # ---- interleaved emission ----
        xs0 = emit_kv_strip(0)
        emit_q_strip(0, xs_reuse=xs0)
        emit_kv_strip(1)
        ps_o = [pso.tile([D + 1, SP], F32, tag=f"ps_o{h}", name=f"ps_o{h}_0")
                for h in range(HPC)]
        pend = []
        for mk in range(0, 8):
            emit_B(0, mk, pend)
        nc.sync.dma_start(wo_sb[:], woT[:])
        emit_kv_strip(2)
        for mk in range(8, 12):
            emit_B(0, mk, pend)
        emit_kv_strip(3)
        emit_q_strip(1)
        for mk in range(12, 16):
            emit_B(0, mk, pend)
        emit_kv_strip(4)
        for mk in range(16, 20):
            emit_B(0, mk, pend)
        emit_kv_strip(5)
        for mk in range(20, 24):
            emit_B(0, mk, pend)
        for s in range(1, NSP):
            ps_o_prev, pend_prev = ps_o, pend
            ps_o = [pso.tile([D + 1, SP], F32, tag=f"ps_o{h}", name=f"ps_o{h}_{s}")
                    for h in range(HPC)]
            pend = []
            for mk in range(0, 4):
                emit_B(s, mk, pend)
            for args in pend_prev:
                emit_pv(*args)
            parts = emit_ep_head(s - 1, ps_o_prev)
            emit_ep_tail(s - 1, parts)
            for mk in range(4, 8):
                emit_B(s, mk, pend)
            if s + 1 < NSP:
                emit_q_strip(s + 1)
            for mk in range(8, NT):
                emit_B(s, mk, pend)
        for args in pend:
            emit_pv(*args)
        parts = emit_ep_head(NSP - 1, ps_o)
        emit_ep_tail(NSP - 1, parts, last=True)

    nc.compile()
    return nc


_PROGRAM = None
LAST_RESULTS = None


def _get_program():
    global _PROGRAM
    if _PROGRAM is None:
        _PROGRAM = build_program()
    return _PROGRAM


def _softplus(x):
    x = np.asarray(x, np.float32)
    return np.logaddexp(0.0, x).astype(np.float32)


def host_prep(inputs):
    x = np.asarray(inputs["x"], np.float32)
    edge_index = np.asarray(inputs["edge_index"])
    edge_type = np.asarray(inputs["edge_type"])
    etw = np.asarray(inputs["edge_type_weights"], np.float32)

    def f32(k):
        return np.asarray(inputs[k], np.float32)

    # compose the two linear layers: q2 = x @ (wiq@wq).T + (wiq@bq + biq)
    WQ = f32("wiq") @ f32("wq")
    bQ = f32("wiq") @ f32("bq") + f32("biq")
    WK = f32("wik") @ f32("wk")
    WV = f32("wiv") @ f32("wv")
    bV = f32("wiv") @ f32("bv") + f32("biv")
    wo = f32("wo")
    bo = f32("bo")
    # bk cancels in softmax; bv contributes exactly bV @ wo.T (attn rows sum
    # to 1), folded into the host-side output bias.
    y_base = (bV @ wo.T + bo).astype(np.float32)

    # multiplicative mask, transposed: maskT[m, n] = exp(add_mask[n, m])
    w = _softplus(etw)
    M = np.zeros((N, N), np.float32)
    src, dst = edge_index[0], edge_index[1]
    ew = np.exp(w).astype(np.float32)
    M[src, dst] = ew[edge_type - 1]            # last write wins, like jax .at[].set
    diag = np.diagonal(M).copy()
    didx = np.arange(N)
    M[didx, didx] = np.where(diag == 0.0, ew[3], diag)
    maskT = np.ascontiguousarray(M.T).astype(np.float16)

    xT = np.ascontiguousarray(x.T).astype(ml_dtypes.bfloat16)

    bf = ml_dtypes.bfloat16
    in_maps = []
    for c in range(NCORES):
        rs = slice(c * CW, (c + 1) * CW)
        in_maps.append({
            "xT": xT,
            "maskT": maskT,
            "wqT": np.ascontiguousarray(WQ[rs].T).astype(bf),
            "wkT": np.ascontiguousarray(WK[rs].T).astype(bf),
            "wvT": np.ascontiguousarray(WV[rs].T).astype(bf),
            "bq": np.ascontiguousarray(bQ[rs][None, :]).astype(bf),
            "woT": np.ascontiguousarray(wo[:, rs].T),
        })
    return in_maps, y_base


def kernel(**inputs) -> np.ndarray:
    global LAST_RESULTS
    in_maps, y_base = host_prep(inputs)
    nc = _get_program()
    trace = bool(os.environ.get("KERNEL_TRACE"))
    res = run_bass_kernel_spmd(nc, in_maps, list(range(NCORES)), trace=trace)
    LAST_RESULTS = res
    y = y_base[None, :].astype(np.float32).repeat(N, axis=0)
    for c in range(NCORES):
        y += res.results[c]["ypart"].astype(np.float32)
    return y


# revision 24
# speedup vs baseline: 1.2416x; 1.0083x over previous
"""Sparse multi-head attention (nn_MultiHeadAttention_44332652429419) on 8 trn2 cores.

Strategy v2 (tensor-parallel over H=16 heads, 2 heads per core, all-bf16 PE):
  Host: compose the stacked linears into one weight per tensor (bf16);
        drop bk (cancels in softmax) and bv (folds into the output bias,
        since attention rows sum to 1); build the multiplicative mask
        exp(additive) transposed, f16; transpose x (bf16).
  Device (per core, SPMD with per-core weight slices):
    q2T/k2T/v2T = W_c @ x.T (+ bq via ones-row)   [128, 3072] bf16
    per (strip of 512 queries, key tile mk):
      scoresT[h] = k2z_h.T @ q2s  -> paired PSUM [128, 2, 512]
      p = exp(scores * 1/8)       one ACT op over both heads (1024 free)
      pm = p * maskT tile         one DVE op, 16-bit (mask broadcast over h)
      ps_o[h] += vaug_h @ pm_h    PV accumulation over mk (ones-col rowsum)
    epilogue per strip: normalize by rowsum, out_proj partial -> ypart bf16
  Host: y = sum_c ypart_c + (bv_eff @ wo.T + bo)
"""
import os
import sys

sys.path.insert(0, "/opt/trn_rl_repo")

import numpy as np
import ml_dtypes
from contextlib import ExitStack

import concourse.bass as bass
import concourse.bacc as bacc
import concourse.mybir as mybir
import concourse.tile as tile
from concourse.bass_utils import run_bass_kernel_spmd
from concourse.masks import make_identity

F32 = mybir.dt.float32
F32R = mybir.dt.float32r
BF16 = mybir.dt.bfloat16
F16 = mybir.dt.float16
AF = mybir.ActivationFunctionType
ALU = mybir.AluOpType

N = 3072
IN_F = 1024
OUT_F = 1024
H = 16
D = 64
NCORES = 8
HPC = H // NCORES            # heads per core = 2
CW = HPC * D                 # per-core width = 128
P = 128
NT = N // P                  # 24 key tiles
KT = IN_F // P               # 8 contraction tiles
SP = 512                     # strip width (queries/nodes)
NSP = N // SP                # 6 strips
SCALE = 1.0 / 8.0            # 1/sqrt(D)


def build_program():
    nc = bacc.Bacc()
    xT = nc.declare_dram_parameter("xT", [IN_F, N], BF16, isOutput=False)
    maskT = nc.declare_dram_parameter("maskT", [N, N], F16, isOutput=False)
    wqT = nc.declare_dram_parameter("wqT", [IN_F, CW], BF16, isOutput=False)
    wkT = nc.declare_dram_parameter("wkT", [IN_F, CW], BF16, isOutput=False)
    wvT = nc.declare_dram_parameter("wvT", [IN_F, CW], BF16, isOutput=False)
    bq = nc.declare_dram_parameter("bq", [1, CW], BF16, isOutput=False)
    woT = nc.declare_dram_parameter("woT", [CW, OUT_F], F32R, isOutput=False)
    ypart = nc.declare_dram_parameter("ypart", [N, OUT_F], BF16, isOutput=True)

    with tile.TileContext(nc) as tc, ExitStack() as ctx:
        cst = ctx.enter_context(tc.tile_pool(name="cst", bufs=1))
        lp = ctx.enter_context(tc.tile_pool(name="lp", bufs=3))       # xs strips
        mtp = ctx.enter_context(tc.tile_pool(name="mtp", bufs=8))     # mask tiles
        ppq = ctx.enter_context(tc.tile_pool(name="ppq", bufs=4))     # p/pm pairs
        ep = ctx.enter_context(tc.tile_pool(name="ep", bufs=2))       # epilogue
        # PSUM: spair 2x2 banks + pso 2 banks + pwork 2x1 banks = 8
        spair = ctx.enter_context(tc.tile_pool(name="spair", bufs=2, space="PSUM"))
        pso = ctx.enter_context(tc.tile_pool(name="pso", bufs=1, space="PSUM"))
        pwork = ctx.enter_context(tc.tile_pool(name="pwork", bufs=2, space="PSUM"))

        ident = cst.tile([P, P], F32)
        make_identity(nc, ident)

        ones_row = cst.tile([1, SP], BF16)
        nc.vector.memset(ones_row[:], 1.0)
        zero_col = cst.tile([P, 1], BF16)
        nc.vector.memset(zero_col[:], 0.0)
        one_col = cst.tile([P, 1], BF16)
        nc.vector.memset(one_col[:], 1.0)

        # persistent activations (bf16)
        q2s = [cst.tile([P, SP], BF16, tag=f"q2s{s}", name=f"q2s{s}")
               for s in range(NSP)]
        k2zs = [[cst.tile([P, SP], BF16, tag=f"k2z{h}_{s}", name=f"k2z{h}_{s}")
                 for s in range(NSP)] for h in range(HPC)]
        vaug = [cst.tile([P, NT, D + 1], BF16, tag=f"vaug{h}", name=f"vaug{h}")
                for h in range(HPC)]
        attn_s = [cst.tile([P, SP], F32R, tag=f"attn{s}", name=f"attn{s}")
                  for s in range(NSP)]
        for h in range(HPC):
            nc.vector.tensor_copy(vaug[h][:, :, D:D + 1],
                                  one_col[:, 0:1, None].to_broadcast([P, NT, 1]))
            osl = slice((1 - h) * D, (2 - h) * D)   # dead half of k2z[h]
            for s in range(NSP):
                nc.vector.tensor_copy(k2zs[h][s][osl, :],
                                      zero_col[osl, 0:1].to_broadcast([D, SP]))

        # weights
        wq_sb = cst.tile([P, KT, CW], BF16)
        for _h2 in range(2):
            nc.sync.dma_start(
                wq_sb[:, 4 * _h2:4 * _h2 + 4, :],
                wqT.rearrange("(k p) m -> p k m", p=P)[:, 4 * _h2:4 * _h2 + 4, :])
        wk_sb = cst.tile([P, KT, CW], BF16)
        for _h2 in range(2):
            nc.sync.dma_start(
                wk_sb[:, 4 * _h2:4 * _h2 + 4, :],
                wkT.rearrange("(k p) m -> p k m", p=P)[:, 4 * _h2:4 * _h2 + 4, :])
        wv_sb = cst.tile([P, KT, CW], BF16)
        for _h2 in range(2):
            nc.sync.dma_start(
                wv_sb[:, 4 * _h2:4 * _h2 + 4, :],
                wvT.rearrange("(k p) m -> p k m", p=P)[:, 4 * _h2:4 * _h2 + 4, :])
        wo_sb = cst.tile([P, OUT_F], F32R)
        bq_sb = cst.tile([1, CW], BF16)
        nc.sync.dma_start(bq_sb[:], bq[:])

        # ---- phase A: k/v projections (needed progressively by all strips);
        #      q-proj is per-strip and deferrable to just before B(s) ----
        def emit_q_strip(s, xs_reuse=None):
            if xs_reuse is None:
                xq_c = [lp.tile([P, 2, SP], BF16, tag=f"xq_c{c}", name=f"xq_c{c}")
                        for c in range(KT // 2)]
                for c in range(KT // 2):
                    nc.sync.dma_start(
                        xq_c[c][:],
                        xT.rearrange("(k p) n -> p k n", p=P)[:, 2 * c:2 * c + 2,
                                                              s * SP:(s + 1) * SP])
            else:
                xq_c = xs_reuse
            ps = pwork.tile([P, SP], F32, tag="pw", name="ps_q")
            for k in range(KT):
                nc.tensor.matmul(ps[:], wq_sb[:, k, :], xq_c[k // 2][:, k % 2, :],
                                 start=(k == 0), stop=False)
            nc.tensor.matmul(ps[:], bq_sb[:], ones_row[:],
                             start=False, stop=True)
            nc.vector.tensor_copy(q2s[s][:], ps[:])

        def emit_kv_strip(s):
            xs_c = [lp.tile([P, 2, SP], BF16, tag=f"xs_c{c}", name=f"xs_c{c}")
                    for c in range(KT // 2)]
            for c in range(KT // 2):
                if s == 0:
                    for j in range(2):
                        nc.sync.dma_start(
                            xs_c[c][:, j, :],
                            xT.rearrange("(k p) n -> p k n", p=P)[:, 2 * c + j,
                                                                  0:SP])
                else:
                    nc.sync.dma_start(
                        xs_c[c][:],
                        xT.rearrange("(k p) n -> p k n", p=P)[:, 2 * c:2 * c + 2,
                                                              s * SP:(s + 1) * SP])

            def xsk(k):
                return xs_c[k // 2][:, k % 2, :]

            # k-proj (no bias)
            ps = pwork.tile([P, SP], F32, tag="pw", name="ps_k")
            for k in range(KT):
                nc.tensor.matmul(ps[:], wk_sb[:, k, :], xsk(k),
                                 start=(k == 0), stop=(k == KT - 1))
            for h in range(HPC):
                hsl = slice(h * D, (h + 1) * D)
                nc.vector.tensor_copy(k2zs[h][s][hsl, :], ps[hsl, :])
            # v-proj (no bias), then transpose into vaug
            ps = pwork.tile([P, SP], F32, tag="pw", name="ps_v")
            for k in range(KT):
                nc.tensor.matmul(ps[:], wv_sb[:, k, :], xsk(k),
                                 start=(k == 0), stop=(k == KT - 1))
            v2Ts = lp.tile([P, SP], F32, tag="v2Ts", name="v2Ts")
            nc.vector.tensor_copy(v2Ts[:], ps[:])
            for b in range(SP // P):
                t = s * (SP // P) + b
                ps_t = pwork.tile([P, SP], F32, tag="pw", name="ps_t")
                nc.tensor.transpose(ps_t[:, 0:P], v2Ts[:, b * P:(b + 1) * P],
                                    ident[:])
                for h in range(HPC):
                    nc.vector.tensor_copy(vaug[h][:, t, 0:D],
                                          ps_t[:, h * D:h * D + D])
            return xs_c

        # ---- phase B: one (strip, key-tile) step, PV deferred via pend ----
        def emit_pv(ps_o_t, mk, pm):
            for h in range(HPC):
                nc.tensor.matmul(ps_o_t[h][:], vaug[h][:, mk, :], pm[:, h, :],
                                 start=(mk == 0), stop=(mk == NT - 1))

        def emit_B(s, mk, pend):
            mt = mtp.tile([P, SP], F16, tag="mt", name="mt")
            nc.sync.dma_start(mt[:], maskT[mk * P:(mk + 1) * P,
                                           s * SP:(s + 1) * SP])
            sp_ = spair.tile([P, HPC, SP], F32, tag="sp", name="sp")
            for h in range(HPC):
                nc.tensor.matmul(sp_[:, h, :],
                                 k2zs[h][mk // 4][:, (mk % 4) * P:(mk % 4 + 1) * P],
                                 q2s[s][:], start=True, stop=True)
            p_ = ppq.tile([P, HPC, SP], BF16, tag="p", name="p")
            nc.scalar.activation(p_[:], sp_[:], AF.Exp, scale=SCALE)
            pm = ppq.tile([P, HPC, SP], BF16, tag="pm", name="pm")
            nc.vector.tensor_tensor(pm[:], p_[:],
                                    mt[:, None, :].to_broadcast([P, HPC, SP]),
                                    ALU.mult)
            pend.append((ps_o, mk, pm))
            if len(pend) > 3:
                emit_pv(*pend.pop(0))

        # ---- epilogue, split: head frees PSUM early; tail does out_proj ----
        def emit_ep_head(s, ps_o_s):
            osbs = []
            for h in range(HPC):
                osb = ep.tile([D + 1, SP], F32, tag=f"osb{h}", name=f"osb{h}")
                nc.vector.tensor_copy(osb[:], ps_o_s[h][:])
                osbs.append(osb)
            parts = []
            for h in range(HPC):
                osb = osbs[h]
                zrow = ep.tile([1, SP], F32, tag="zrow", name=f"zrow{h}")
                nc.vector.tensor_copy(zrow[:], osb[D:D + 1, :])
                recip = ep.tile([1, SP], F32, tag="recip", name=f"recip{h}")
                nc.vector.reciprocal_approx_fast(recip[:], zrow[:])
                bc = ep.tile([D, SP], F32, tag=f"bc{h}", name=f"bc{h}")
                nc.gpsimd.partition_broadcast(bc[:], recip[:])
                parts.append((osb, bc))
            return parts

        def emit_ep_tail(s, parts, last=False):
            for h in range(HPC):
                osb, bc = parts[h]
                nc.vector.tensor_tensor(attn_s[s][h * D:(h + 1) * D, :],
                                        osb[0:D, :], bc[:], ALU.mult)
            for b in range(SP // P):
                t = s * (SP // P) + b
                ys = ep.tile([P, OUT_F], BF16, tag="ys", name="ys")
                for f in range(OUT_F // SP):
                    ps_y = pwork.tile([P, SP], F32, tag="pw", name="ps_y")
                    nc.tensor.matmul(ps_y[:],
                                     attn_s[s][:, b * P:(b + 1) * P],
                                     wo_sb[:, f * SP:(f + 1) * SP],
                                     start=True, stop=True)
                    if (b + f) % 2 == 0:
                        nc.scalar.activation(ys[:, f * SP:(f + 1) * SP],
                                             ps_y[:], AF.Copy)
                    else:
                        nc.vector.tensor_copy(ys[:, f * SP:(f + 1) * SP],
                                              ps_y[:])
                    if last:
                        nc.sync.dma_start(
                            ypart[t * P:(t + 1) * P, f * SP:(f + 1) * SP],
                            ys[:, f * SP:(f + 1) * SP])
                if not last:
                    nc.sync.dma_start(ypart[t * P:(t + 1) * P, :], ys[:])

        # ---- interleaved emission ----
        xs0 = emit_kv_strip(0)
        emit_q_strip(0, xs_reuse=xs0)
        emit_kv_strip(1)
        ps_o = [pso.tile([D + 1, SP], F32, tag=f"ps_o{h}", name=f"ps_o{h}_0")
                for h in range(HPC)]
        pend = []
        for mk in range(0, 8):
            emit_B(0, mk, pend)
        nc.sync.dma_start(wo_sb[:], woT[:])
        emit_kv_strip(2)
        for mk in range(8, 12):
            emit_B(0, mk, pend)
        emit_kv_strip(3)
        emit_q_strip(1)
        for mk in range(12, 16):
            emit_B(0, mk, pend)
        emit_kv_strip(4)
        for mk in range(16, 20):
            emit_B(0, mk, pend)
        emit_kv_strip(5)
        for mk in range(20, 24):
            emit_B(0, mk, pend)
        for s in range(1, NSP):
            ps_o_prev, pend_prev = ps_o, pend
            ps_o = [pso.tile([D + 1, SP], F32, tag=f"ps_o{h}", name=f"ps_o{h}_{s}")
                    for h in range(HPC)]
            pend = []
            for mk in range(0, 4):
                emit_B(s, mk, pend)
            for args in pend_prev:
                emit_pv(*args)
            parts = emit_ep_head(s - 1, ps_o_prev)
            emit_ep_tail(s - 1, parts)
            for mk in range(4, 8):
                emit_B(s, mk, pend)
            if s + 1 < NSP:
                emit_q_strip(s + 1)
            for mk in range(8, NT):
                emit_B(s, mk, pend)
        for args in pend:
            emit_pv(*args)
        parts = emit_ep_head(NSP - 1, ps_o)
        emit_ep_tail(NSP - 1, parts, last=True)

    nc.compile()
    return nc


_PROGRAM = None
LAST_RESULTS = None


def _get_program():
    global _PROGRAM
    if _PROGRAM is None:
        _PROGRAM = build_program()
    return _PROGRAM


def _softplus(x):
    x = np.asarray(x, np.float32)
    return np.logaddexp(0.0, x).astype(np.float32)


def host_prep(inputs):
    x = np.asarray(inputs["x"], np.float32)
    edge_index = np.asarray(inputs["edge_index"])
    edge_type = np.asarray(inputs["edge_type"])
    etw = np.asarray(inputs["edge_type_weights"], np.float32)

    def f32(k):
        return np.asarray(inputs[k], np.float32)

    # compose the two linear layers: q2 = x @ (wiq@wq).T + (wiq@bq + biq)
    WQ = f32("wiq") @ f32("wq")
    bQ = f32("wiq") @ f32("bq") + f32("biq")
    WK = f32("wik") @ f32("wk")
    WV = f32("wiv") @ f32("wv")
    bV = f32("wiv") @ f32("bv") + f32("biv")
    wo = f32("wo")
    bo = f32("bo")
    # bk cancels in softmax; bv contributes exactly bV @ wo.T (attn rows sum
    # to 1), folded into the host-side output bias.
    y_base = (bV @ wo.T + bo).astype(np.float32)

    # multiplicative mask, transposed: maskT[m, n] = exp(add_mask[n, m])
    w = _softplus(etw)
    M = np.zeros((N, N), np.float32)
    src, dst = edge_index[0], edge_index[1]
    ew = np.exp(w).astype(np.float32)
    M[src, dst] = ew[edge_type - 1]            # last write wins, like jax .at[].set
    diag = np.diagonal(M).copy()
    didx = np.arange(N)
    M[didx, didx] = np.where(diag == 0.0, ew[3], diag)
    maskT = np.ascontiguousarray(M.T).astype(np.float16)

    xT = np.ascontiguousarray(x.T).astype(ml_dtypes.bfloat16)

    bf = ml_dtypes.bfloat16
    in_maps = []
    for c in range(NCORES):
        rs = slice(c * CW, (c + 1) * CW)
        in_maps.append({
            "xT": xT,
            "maskT": maskT,
            "wqT": np.ascontiguousarray(WQ[rs].T).astype(bf),
            "wkT": np.ascontiguousarray(WK[rs].T).astype(bf),
            "wvT": np.ascontiguousarray(WV[rs].T).astype(bf),
            "bq": np.ascontiguousarray(bQ[rs][None, :]).astype(bf),
            "woT": np.ascontiguousarray(wo[:, rs].T),
        })
    return in_maps, y_base


def kernel(**inputs) -> np.ndarray:
    global LAST_RESULTS
    in_maps, y_base = host_prep(inputs)
    nc = _get_program()
    trace = bool(os.environ.get("KERNEL_TRACE"))
    res = run_bass_kernel_spmd(nc, in_maps, list(range(NCORES)), trace=trace)
    LAST_RESULTS = res
    y = y_base[None, :].astype(np.float32).repeat(N, axis=0)
    for c in range(NCORES):
        y += res.results[c]["ypart"].astype(np.float32)
    return y


# revision 25
# speedup vs baseline: 1.2879x; 1.0373x over previous
"""Sparse multi-head attention (nn_MultiHeadAttention_44332652429419) on 8 trn2 cores.

Strategy v2 (tensor-parallel over H=16 heads, 2 heads per core, all-bf16 PE):
  Host: compose the stacked linears into one weight per tensor (bf16);
        drop bk (cancels in softmax) and bv (folds into the output bias,
        since attention rows sum to 1); build the multiplicative mask
        exp(additive) transposed, f16; transpose x (bf16).
  Device (per core, SPMD with per-core weight slices):
    q2T/k2T/v2T = W_c @ x.T (+ bq via ones-row)   [128, 3072] bf16
    per (strip of 512 queries, key tile mk):
      scoresT[h] = k2z_h.T @ q2s  -> paired PSUM [128, 2, 512]
      p = exp(scores * 1/8)       one ACT op over both heads (1024 free)
      pm = p * maskT tile         one DVE op, 16-bit (mask broadcast over h)
      ps_o[h] += vaug_h @ pm_h    PV accumulation over mk (ones-col rowsum)
    epilogue per strip: normalize by rowsum, out_proj partial -> ypart bf16
  Host: y = sum_c ypart_c + (bv_eff @ wo.T + bo)
"""
import os
import sys

sys.path.insert(0, "/opt/trn_rl_repo")

import numpy as np
import ml_dtypes
from contextlib import ExitStack

import concourse.bass as bass
import concourse.bacc as bacc
import concourse.mybir as mybir
import concourse.tile as tile
from concourse.bass_utils import run_bass_kernel_spmd
from concourse.masks import make_identity

F32 = mybir.dt.float32
F32R = mybir.dt.float32r
BF16 = mybir.dt.bfloat16
F16 = mybir.dt.float16
AF = mybir.ActivationFunctionType
ALU = mybir.AluOpType

N = 3072
IN_F = 1024
OUT_F = 1024
H = 16
D = 64
NCORES = 8
HPC = H // NCORES            # heads per core = 2
CW = HPC * D                 # per-core width = 128
P = 128
NT = N // P                  # 24 key tiles
KT = IN_F // P               # 8 contraction tiles
SP = 512                     # strip width (queries/nodes)
NSP = N // SP                # 6 strips
SCALE = 1.0 / 8.0            # 1/sqrt(D)


def build_program():
    nc = bacc.Bacc()
    xT = nc.declare_dram_parameter("xT", [IN_F, N], BF16, isOutput=False)
    maskT = nc.declare_dram_parameter("maskT", [N, N], F16, isOutput=False)
    wqT = nc.declare_dram_parameter("wqT", [IN_F, CW], BF16, isOutput=False)
    wkT = nc.declare_dram_parameter("wkT", [IN_F, CW], BF16, isOutput=False)
    wvT = nc.declare_dram_parameter("wvT", [IN_F, CW], BF16, isOutput=False)
    bq = nc.declare_dram_parameter("bq", [1, CW], BF16, isOutput=False)
    woT = nc.declare_dram_parameter("woT", [CW, OUT_F], F32R, isOutput=False)
    ypart = nc.declare_dram_parameter("ypart", [N, OUT_F], BF16, isOutput=True)

    with tile.TileContext(nc) as tc, ExitStack() as ctx:
        cst = ctx.enter_context(tc.tile_pool(name="cst", bufs=1))
        lp = ctx.enter_context(tc.tile_pool(name="lp", bufs=3))       # xs strips
        mtp = ctx.enter_context(tc.tile_pool(name="mtp", bufs=8))     # mask tiles
        ppq = ctx.enter_context(tc.tile_pool(name="ppq", bufs=4))     # p/pm pairs
        ep = ctx.enter_context(tc.tile_pool(name="ep", bufs=2))       # epilogue
        # PSUM: spair 2x2 banks + pso 2 banks + pwork 2x1 banks = 8
        spair = ctx.enter_context(tc.tile_pool(name="spair", bufs=2, space="PSUM"))
        pso = ctx.enter_context(tc.tile_pool(name="pso", bufs=1, space="PSUM"))
        pwork = ctx.enter_context(tc.tile_pool(name="pwork", bufs=2, space="PSUM"))

        ident = cst.tile([P, P], F32)
        make_identity(nc, ident)

        ones_row = cst.tile([1, SP], BF16)
        nc.vector.memset(ones_row[:], 1.0)
        zero_col = cst.tile([P, 1], BF16)
        nc.vector.memset(zero_col[:], 0.0)
        one_col = cst.tile([P, 1], BF16)
        nc.vector.memset(one_col[:], 1.0)

        # persistent activations (bf16)
        q2s = [cst.tile([P, SP], BF16, tag=f"q2s{s}", name=f"q2s{s}")
               for s in range(NSP)]
        k2zs = [[cst.tile([P, SP], BF16, tag=f"k2z{h}_{s}", name=f"k2z{h}_{s}")
                 for s in range(NSP)] for h in range(HPC)]
        vaug = [cst.tile([P, NT, D + 1], BF16, tag=f"vaug{h}", name=f"vaug{h}")
                for h in range(HPC)]
        attn_s = [cst.tile([P, SP], F32R, tag=f"attn{s}", name=f"attn{s}")
                  for s in range(NSP)]
        for h in range(HPC):
            nc.vector.tensor_copy(vaug[h][:, :, D:D + 1],
                                  one_col[:, 0:1, None].to_broadcast([P, NT, 1]))
            osl = slice((1 - h) * D, (2 - h) * D)   # dead half of k2z[h]
            for s in range(NSP):
                nc.vector.tensor_copy(k2zs[h][s][osl, :],
                                      zero_col[osl, 0:1].to_broadcast([D, SP]))

        # PE p-state warmup during the startup DMA window: 24 WAW-chained
        # dummy matmuls into one dedicated PSUM tile (pso ring, bufs=1, so
        # no ring-parity shift for any later allocation)
        warm_w = cst.tile([P, P], BF16)
        nc.vector.memset(warm_w[:], 0.0)
        warm_x = cst.tile([P, SP], BF16)
        nc.vector.memset(warm_x[:], 0.0)
        wps = pso.tile([D + 1, SP], F32, tag="ps_o0", name="warm_ps")
        for _wi in range(24):
            nc.tensor.matmul(wps[:, 0:SP], warm_w[:, 0:D + 1], warm_x[:],
                             start=True, stop=True)

        # weights
        wq_sb = cst.tile([P, KT, CW], BF16)
        for _h2 in range(2):
            nc.sync.dma_start(
                wq_sb[:, 4 * _h2:4 * _h2 + 4, :],
                wqT.rearrange("(k p) m -> p k m", p=P)[:, 4 * _h2:4 * _h2 + 4, :])
        wk_sb = cst.tile([P, KT, CW], BF16)
        for _h2 in range(2):
            nc.sync.dma_start(
                wk_sb[:, 4 * _h2:4 * _h2 + 4, :],
                wkT.rearrange("(k p) m -> p k m", p=P)[:, 4 * _h2:4 * _h2 + 4, :])
        wv_sb = cst.tile([P, KT, CW], BF16)
        for _h2 in range(2):
            nc.sync.dma_start(
                wv_sb[:, 4 * _h2:4 * _h2 + 4, :],
                wvT.rearrange("(k p) m -> p k m", p=P)[:, 4 * _h2:4 * _h2 + 4, :])
        wo_sb = cst.tile([P, OUT_F], F32R)
        bq_sb = cst.tile([1, CW], BF16)
        nc.sync.dma_start(bq_sb[:], bq[:])

        # ---- phase A: k/v projections (needed progressively by all strips);
        #      q-proj is per-strip and deferrable to just before B(s) ----
        def emit_q_strip(s, xs_reuse=None):
            if xs_reuse is None:
                xq_c = [lp.tile([P, 2, SP], BF16, tag=f"xq_c{c}", name=f"xq_c{c}")
                        for c in range(KT // 2)]
                for c in range(KT // 2):
                    nc.sync.dma_start(
                        xq_c[c][:],
                        xT.rearrange("(k p) n -> p k n", p=P)[:, 2 * c:2 * c + 2,
                                                              s * SP:(s + 1) * SP])
            else:
                xq_c = xs_reuse
            ps = pwork.tile([P, SP], F32, tag="pw", name="ps_q")
            for k in range(KT):
                nc.tensor.matmul(ps[:], wq_sb[:, k, :], xq_c[k // 2][:, k % 2, :],
                                 start=(k == 0), stop=False)
            nc.tensor.matmul(ps[:], bq_sb[:], ones_row[:],
                             start=False, stop=True)
            nc.vector.tensor_copy(q2s[s][:], ps[:])

        def emit_kv_strip(s):
            xs_c = [lp.tile([P, 2, SP], BF16, tag=f"xs_c{c}", name=f"xs_c{c}")
                    for c in range(KT // 2)]
            for c in range(KT // 2):
                if s == 0:
                    for j in range(2):
                        nc.sync.dma_start(
                            xs_c[c][:, j, :],
                            xT.rearrange("(k p) n -> p k n", p=P)[:, 2 * c + j,
                                                                  0:SP])
                else:
                    nc.sync.dma_start(
                        xs_c[c][:],
                        xT.rearrange("(k p) n -> p k n", p=P)[:, 2 * c:2 * c + 2,
                                                              s * SP:(s + 1) * SP])

            def xsk(k):
                return xs_c[k // 2][:, k % 2, :]

            # k-proj (no bias)
            ps = pwork.tile([P, SP], F32, tag="pw", name="ps_k")
            for k in range(KT):
                nc.tensor.matmul(ps[:], wk_sb[:, k, :], xsk(k),
                                 start=(k == 0), stop=(k == KT - 1))
            for h in range(HPC):
                hsl = slice(h * D, (h + 1) * D)
                nc.vector.tensor_copy(k2zs[h][s][hsl, :], ps[hsl, :])
            # v-proj (no bias), then transpose into vaug
            ps = pwork.tile([P, SP], F32, tag="pw", name="ps_v")
            for k in range(KT):
                nc.tensor.matmul(ps[:], wv_sb[:, k, :], xsk(k),
                                 start=(k == 0), stop=(k == KT - 1))
            v2Ts = lp.tile([P, SP], F32, tag="v2Ts", name="v2Ts")
            nc.vector.tensor_copy(v2Ts[:], ps[:])
            for b in range(SP // P):
                t = s * (SP // P) + b
                ps_t = pwork.tile([P, SP], F32, tag="pw", name="ps_t")
                nc.tensor.transpose(ps_t[:, 0:P], v2Ts[:, b * P:(b + 1) * P],
                                    ident[:])
                for h in range(HPC):
                    nc.vector.tensor_copy(vaug[h][:, t, 0:D],
                                          ps_t[:, h * D:h * D + D])
            return xs_c

        # ---- phase B: one (strip, key-tile) step, PV deferred via pend ----
        def emit_pv(ps_o_t, mk, pm):
            for h in range(HPC):
                nc.tensor.matmul(ps_o_t[h][:], vaug[h][:, mk, :], pm[:, h, :],
                                 start=(mk == 0), stop=(mk == NT - 1))

        def emit_B(s, mk, pend):
            mt = mtp.tile([P, SP], F16, tag="mt", name="mt")
            nc.sync.dma_start(mt[:], maskT[mk * P:(mk + 1) * P,
                                           s * SP:(s + 1) * SP])
            sp_ = spair.tile([P, HPC, SP], F32, tag="sp", name="sp")
            for h in range(HPC):
                nc.tensor.matmul(sp_[:, h, :],
                                 k2zs[h][mk // 4][:, (mk % 4) * P:(mk % 4 + 1) * P],
                                 q2s[s][:], start=True, stop=True)
            p_ = ppq.tile([P, HPC, SP], BF16, tag="p", name="p")
            nc.scalar.activation(p_[:], sp_[:], AF.Exp, scale=SCALE)
            pm = ppq.tile([P, HPC, SP], BF16, tag="pm", name="pm")
            nc.vector.tensor_tensor(pm[:], p_[:],
                                    mt[:, None, :].to_broadcast([P, HPC, SP]),
                                    ALU.mult)
            pend.append((ps_o, mk, pm))
            if len(pend) > 2:
                emit_pv(*pend.pop(0))

        # ---- epilogue, split: head frees PSUM early; tail does out_proj ----
        def emit_ep_head(s, ps_o_s):
            osbs = []
            for h in range(HPC):
                osb = ep.tile([D + 1, SP], F32, tag=f"osb{h}", name=f"osb{h}")
                nc.vector.tensor_copy(osb[:], ps_o_s[h][:])
                osbs.append(osb)
            parts = []
            for h in range(HPC):
                osb = osbs[h]
                zrow = ep.tile([1, SP], F32, tag="zrow", name=f"zrow{h}")
                nc.vector.tensor_copy(zrow[:], osb[D:D + 1, :])
                recip = ep.tile([1, SP], F32, tag="recip", name=f"recip{h}")
                nc.vector.reciprocal_approx_fast(recip[:], zrow[:])
                bc = ep.tile([D, SP], F32, tag=f"bc{h}", name=f"bc{h}")
                nc.gpsimd.partition_broadcast(bc[:], recip[:])
                parts.append((osb, bc))
            return parts

        def emit_ep_tail(s, parts, last=False):
            for h in range(HPC):
                osb, bc = parts[h]
                nc.vector.tensor_tensor(attn_s[s][h * D:(h + 1) * D, :],
                                        osb[0:D, :], bc[:], ALU.mult)
            for b in range(SP // P):
                t = s * (SP // P) + b
                ys = ep.tile([P, OUT_F], BF16, tag="ys", name="ys")
                for f in range(OUT_F // SP):
                    ps_y = pwork.tile([P, SP], F32, tag="pw", name="ps_y")
                    nc.tensor.matmul(ps_y[:],
                                     attn_s[s][:, b * P:(b + 1) * P],
                                     wo_sb[:, f * SP:(f + 1) * SP],
                                     start=True, stop=True)
                    if (b + f) % 2 == 0:
                        nc.scalar.activation(ys[:, f * SP:(f + 1) * SP],
                                             ps_y[:], AF.Copy)
                    else:
                        nc.vector.tensor_copy(ys[:, f * SP:(f + 1) * SP],
                                              ps_y[:])
                    if last:
                        nc.sync.dma_start(
                            ypart[t * P:(t + 1) * P, f * SP:(f + 1) * SP],
                            ys[:, f * SP:(f + 1) * SP])
                if not last:
                    nc.sync.dma_start(ypart[t * P:(t + 1) * P, :], ys[:])

        # ---- interleaved emission ----
        xs0 = emit_kv_strip(0)
        emit_q_strip(0, xs_reuse=xs0)
        emit_kv_strip(1)
        ps_o = [pso.tile([D + 1, SP], F32, tag=f"ps_o{h}", name=f"ps_o{h}_0")
                for h in range(HPC)]
        pend = []
        for mk in range(0, 8):
            emit_B(0, mk, pend)
        nc.sync.dma_start(wo_sb[:], woT[:])
        emit_kv_strip(2)
        for mk in range(8, 12):
            emit_B(0, mk, pend)
        emit_kv_strip(3)
        emit_q_strip(1)
        for mk in range(12, 16):
            emit_B(0, mk, pend)
        emit_kv_strip(4)
        for mk in range(16, 20):
            emit_B(0, mk, pend)
        emit_kv_strip(5)
        for mk in range(20, 24):
            emit_B(0, mk, pend)
        for s in range(1, NSP):
            ps_o_prev, pend_prev = ps_o, pend
            ps_o = [pso.tile([D + 1, SP], F32, tag=f"ps_o{h}", name=f"ps_o{h}_{s}")
                    for h in range(HPC)]
            pend = []
            for mk in range(0, 4):
                emit_B(s, mk, pend)
            for args in pend_prev:
                emit_pv(*args)
            parts = emit_ep_head(s - 1, ps_o_prev)
            emit_ep_tail(s - 1, parts)
            for mk in range(4, 8):
                emit_B(s, mk, pend)
            if s + 1 < NSP:
                emit_q_strip(s + 1)
            for mk in range(8, NT):
                emit_B(s, mk, pend)
        for args in pend:
            emit_pv(*args)
        parts = emit_ep_head(NSP - 1, ps_o)
        emit_ep_tail(NSP - 1, parts, last=True)

    nc.compile()
    return nc


_PROGRAM = None
LAST_RESULTS = None


def _get_program():
    global _PROGRAM
    if _PROGRAM is None:
        _PROGRAM = build_program()
    return _PROGRAM


def _softplus(x):
    x = np.asarray(x, np.float32)
    return np.logaddexp(0.0, x).astype(np.float32)


def host_prep(inputs):
    x = np.asarray(inputs["x"], np.float32)
    edge_index = np.asarray(inputs["edge_index"])
    edge_type = np.asarray(inputs["edge_type"])
    etw = np.asarray(inputs["edge_type_weights"], np.float32)

    def f32(k):
        return np.asarray(inputs[k], np.float32)

    # compose the two linear layers: q2 = x @ (wiq@wq).T + (wiq@bq + biq)
    WQ = f32("wiq") @ f32("wq")
    bQ = f32("wiq") @ f32("bq") + f32("biq")
    WK = f32("wik") @ f32("wk")
    WV = f32("wiv") @ f32("wv")
    bV = f32("wiv") @ f32("bv") + f32("biv")
    wo = f32("wo")
    bo = f32("bo")
    # bk cancels in softmax; bv contributes exactly bV @ wo.T (attn rows sum
    # to 1), folded into the host-side output bias.
    y_base = (bV @ wo.T + bo).astype(np.float32)

    # multiplicative mask, transposed: maskT[m, n] = exp(add_mask[n, m])
    w = _softplus(etw)
    M = np.zeros((N, N), np.float32)
    src, dst = edge_index[0], edge_index[1]
    ew = np.exp(w).astype(np.float32)
    M[src, dst] = ew[edge_type - 1]            # last write wins, like jax .at[].set
    diag = np.diagonal(M).copy()
    didx = np.arange(N)
    M[didx, didx] = np.where(diag == 0.0, ew[3], diag)
    maskT = np.ascontiguousarray(M.T).astype(np.float16)

    xT = np.ascontiguousarray(x.T).astype(ml_dtypes.bfloat16)

    bf = ml_dtypes.bfloat16
    in_maps = []
    for c in range(NCORES):
        rs = slice(c * CW, (c + 1) * CW)
        in_maps.append({
            "xT": xT,
            "maskT": maskT,
            "wqT": np.ascontiguousarray(WQ[rs].T).astype(bf),
            "wkT": np.ascontiguousarray(WK[rs].T).astype(bf),
            "wvT": np.ascontiguousarray(WV[rs].T).astype(bf),
            "bq": np.ascontiguousarray(bQ[rs][None, :]).astype(bf),
            "woT": np.ascontiguousarray(wo[:, rs].T),
        })
    return in_maps, y_base


def kernel(**inputs) -> np.ndarray:
    global LAST_RESULTS
    in_maps, y_base = host_prep(inputs)
    nc = _get_program()
    trace = bool(os.environ.get("KERNEL_TRACE"))
    res = run_bass_kernel_spmd(nc, in_maps, list(range(NCORES)), trace=trace)
    LAST_RESULTS = res
    y = y_base[None, :].astype(np.float32).repeat(N, axis=0)
    for c in range(NCORES):
        y += res.results[c]["ypart"].astype(np.float32)
    return y
